# revision 50
# baseline (speedup 1.0000x reference)
"""Trainium2 Bass kernel for the nn_AttentionLayer problem.

Full multi-head attention layer, B=4, L=S=2048, d_model=1024, 16 heads of
dim 64, with the reference's "mix=True" transpose-then-flatten before the
output projection.

Key observation: the mix reshape means output row l' = h*128 + l//16 of each
batch depends ONLY on head h.  So sharding 8 cores as (batch, half-of-heads)
makes every core produce a disjoint, contiguous 1024-row slice of the output
with zero cross-core communication.

Per-core dataflow (all matmul inputs in `mm_dt`, bf16 by default):
  - inputs arrive host-transposed ([D, L]) so the QKV projections contract
    d_model on the partition dim with natural weight layouts
  - v is projected up front into v_aug = [v | ones] s-tiles; the ones
    column makes row 64 of every A@V accumulator the softmax denominator
  - k/q are projected one 128-wide e-chunk (= head pair) at a time; the
    chunk for pair p+1 is drip-fed ("filler" generator) into the PE stream
    while pair p's ACT-bound attention runs, as is pair p-1's output
    projection - the tensor engine fills slack instead of bursting
  - per (pair, l-chunk, s-tile): both heads' scoresT land in one wide
    [128, 1024] PSUM tile via two matmuls on complementary PE row groups
    (true row-packing: K=64 each, concurrent), one wide exp covers both
  - softmax normalize: quick-release copy of each A@V accumulator to SBUF
    (frees the PSUM bank), then reciprocal + gpsimd partition_broadcast +
    multiply, staggered one l-chunk behind the accumulation
  - the normalized attn [64, L] is self-copied (SBUF->SBUF DMA) to
    partitions 64..127 shifted by one position, which makes the output
    projection a clean K=128 matmul against natural Wo row-pair tiles
"""

from collections import deque

import numpy as np

import concourse.bass as bass
import concourse.mybir as mybir
import concourse.tile as tile
from concourse import bacc
from concourse.bass_utils import run_bass_kernel_spmd

F32 = mybir.dt.float32
EXP = mybir.ActivationFunctionType.Exp
IDENT = mybir.ActivationFunctionType.Identity
MULT = mybir.AluOpType.mult
ADD = mybir.AluOpType.add

E = 64          # head dim
J = 16          # mix factor: total heads in the reference model
JE = J * E      # 1024 rows of Wo

MM_DTS = {
    "f32r": mybir.dt.float32r,
    "f32": mybir.dt.float32,
    "bf16": mybir.dt.bfloat16,
}


def build_core_kernel(L=2048, D=1024, NH=8, OUT_D=1024, mm_dt="bf16",
                      taps=False, fill_per_st=3):
    """Builds the per-core Bacc graph (SPMD: all 8 cores run this)."""
    HE = NH * E               # projected width per core
    HEC = HE // 128           # qT/kT tiles (head pairs)
    NHP = NH // 2             # head pairs
    KC = D // 128             # contraction tiles for projections
    R = L // J                # output rows per head
    LCH = min(512, L)         # l-chunk
    NLC = L // LCH
    NST = L // 128            # s-tiles
    DCH = min(512, OUT_D)     # out-proj n-chunk
    NDC = OUT_D // DCH
    SCALE = 1.0 / np.sqrt(E)
    assert L % J == 0 and R <= 128 and HE % 128 == 0

    MDT = MM_DTS[mm_dt]

    nc = bacc.Bacc("TRN2", target_bir_lowering=False, debug=False,
                   enable_asserts=False)

    qT_ext = nc.declare_dram_parameter("qT", [D, L], MDT, isOutput=False)
    kT_ext = nc.declare_dram_parameter("kT", [D, L], MDT, isOutput=False)
    vT_ext = nc.declare_dram_parameter("vT", [D, L], MDT, isOutput=False)
    wq_ext = nc.declare_dram_parameter("wq", [D, HE], MDT, isOutput=False)
    wk_ext = nc.declare_dram_parameter("wk", [D, HE], MDT, isOutput=False)
    wv_ext = nc.declare_dram_parameter("wv", [D, HE], MDT, isOutput=False)
    bq_ext = nc.declare_dram_parameter("bq", [HE], F32, isOutput=False)
    bk_ext = nc.declare_dram_parameter("bk", [HE], F32, isOutput=False)
    bv_ext = nc.declare_dram_parameter("bv", [HE], F32, isOutput=False)
    wo_ext = nc.declare_dram_parameter("wo", [JE, OUT_D], MDT, isOutput=False)
    bo_ext = nc.declare_dram_parameter("bo", [OUT_D], F32, isOutput=False)
    out_ext = nc.declare_dram_parameter("out", [NH * R, OUT_D], F32,
                                        isOutput=True)
    if taps:
        dbg_qT = nc.declare_dram_parameter("dbg_qT", [HEC * 128, L], MDT,
                                           isOutput=True)
        dbg_kT = nc.declare_dram_parameter("dbg_kT", [HEC * 128, L], MDT,
                                           isOutput=True)
        dbg_v = nc.declare_dram_parameter("dbg_v", [NST * 128, NH * (E + 1)],
                                          MDT, isOutput=True)
        dbg_ex = nc.declare_dram_parameter("dbg_ex", [128, LCH], MDT,
                                           isOutput=True)
        dbg_dup = nc.declare_dram_parameter("dbg_dup", [128, L], MDT,
                                            isOutput=True)

    with tile.TileContext(nc) as tc:
        with (
            tc.tile_pool(name="const", bufs=1) as const,
            tc.tile_pool(name="wsl", bufs=2 * KC + 1) as wsl,
            tc.tile_pool(name="qin", bufs=1) as qin,
            tc.tile_pool(name="acts", bufs=1) as acts,
            tc.tile_pool(name="expp", bufs=5) as expp,
            tc.tile_pool(name="attnd", bufs=3) as attnd,
            tc.tile_pool(name="small", bufs=2) as small,
            tc.tile_pool(name="outp", bufs=3) as outp,
        ):
            # ---- constants ----
            bqt = const.tile([128, HEC], F32, tag="bqt")
            nc.sync.dma_start(bqt[:], bq_ext.rearrange("(c p) -> p c", p=128))
            bqs = const.tile([128, HEC], F32, tag="bqs")
            nc.vector.tensor_scalar_mul(bqs[:], bqt[:], float(SCALE))
            bkt = const.tile([128, HEC], F32, tag="bkt")
            nc.sync.dma_start(bkt[:], bk_ext.rearrange("(c p) -> p c", p=128))

            bv_row = const.tile([1, HE], F32, tag="bv_row")
            nc.sync.dma_start(bv_row[:],
                              bv_ext.rearrange("(o he) -> o he", o=1))
            bv_bc = const.tile([128, HE], F32, tag="bv_bc")
            nc.gpsimd.partition_broadcast(bv_bc[:], bv_row[:], channels=128)

            bo_row = const.tile([1, OUT_D], F32, tag="bo_row")
            nc.sync.dma_start(bo_row[:],
                              bo_ext.rearrange("(o d) -> o d", o=1))
            bo_bc = const.tile([128, OUT_D], F32, tag="bo_bc")
            nc.gpsimd.partition_broadcast(bo_bc[:], bo_row[:], channels=128)

            ones_t = const.tile([128, NH], F32, tag="ones_t")
            nc.vector.memset(ones_t[:], 1.0)

            qT_sb = [acts.tile([128, L], MDT, tag=f"qT{i}", name=f"qT_sb{i}")
                     for i in range(HEC)]
            kT_sb = [acts.tile([128, L], MDT, tag=f"kT{i}", name=f"kT_sb{i}")
                     for i in range(HEC)]

            def load_w(w_ext):
                wt = []
                for dt in range(KC):
                    w = wsl.tile([128, HE], MDT, tag="wsl", name="w_t")
                    nc.sync.dma_start(w[:],
                                      w_ext[dt * 128:(dt + 1) * 128, :])
                    wt.append(w)
                return wt

            def load_xin(in_ext, pfx):
                # full-row d-tiles: one big DMA each, live for the phase
                tiles = []
                for dt in range(KC):
                    x = qin.tile([128, L], MDT, tag=f"{pfx}{dt}",
                                 name=f"{pfx}{dt}", bufs=1)
                    nc.sync.dma_start(x[:],
                                      in_ext[dt * 128:(dt + 1) * 128, :])
                    tiles.append(x)
                return tiles

            # ---- k/q inputs first in the DMA queue (the K0/Q0 prologue
            # chains are paced by them); v is projected just-in-time via a
            # generator with small per-tile DMAs dripped into pair0/lc0
            wkt = load_w(wk_ext)
            kin = load_xin(kT_ext, "xk")
            wqt = load_w(wq_ext)
            qin_t = load_xin(qT_ext, "xq")
            wvt = load_w(wv_ext)

            v_aug = []
            for st in range(NST):
                v = acts.tile([128, NH * (E + 1)], MDT, tag=f"vaug{st}",
                              name=f"vaug{st}")
                v_aug.append(v)
                nc.vector.tensor_copy(
                    v.rearrange("p (h u) -> p h u", u=E + 1)[:, :, E:E + 1],
                    ones_t.rearrange("p (h o) -> p h o", o=1))

            # Wo preload after the projection-input DMAs in trace order
            wo_sb = []
            for t in range(JE // 128):
                w = const.tile([128, OUT_D], MDT, tag=f"wo{t}",
                               name=f"wo_sb{t}")
                nc.sync.dma_start(w[:], wo_ext[t * 128:(t + 1) * 128, :])
                wo_sb.append(w)

            # ---- merged attention + drip-fed projections/output ----
            W = 2 * LCH
            with (
                tc.tile_pool(name="pq", bufs=1, space="PSUM") as pq,
                tc.tile_pool(name="psc", bufs=2, space="PSUM") as psc,
                tc.tile_pool(name="pacc", bufs=3, space="PSUM") as pacc,
            ):
                def vchain_gen():
                    for st in range(NST):
                        psv = pacc.tile([128, HE], F32, tag="acc",
                                        name="psv")
                        for dt in range(KC):
                            vin = qin.tile([128, 128], MDT, tag="vin",
                                           name="vin", bufs=4)
                            nc.sync.dma_start(
                                vin[:],
                                vT_ext[dt * 128:(dt + 1) * 128,
                                       st * 128:(st + 1) * 128])
                            nc.tensor.matmul(psv[:], vin[:], wvt[dt][:],
                                             start=(dt == 0),
                                             stop=(dt == KC - 1))
                        nc.vector.tensor_add(
                            v_aug[st].rearrange("p (h u) -> p h u",
                                                u=E + 1)[:, :, 0:E],
                            psv.rearrange("p (h e) -> p h e", e=E)[:],
                            bv_bc.rearrange("p (h e) -> p h e", e=E)[:])
                        yield

                def proj_gen(wt, xin, dest, hp, is_q, pool=None,
                             ptag="pq"):
                    """k/q projection of e-chunk hp, one yield per step."""
                    pool = pool or pq
                    for lc in range(NLC):
                        psq = pool.tile([128, LCH], F32, tag=ptag,
                                        name="psq")
                        for dt in range(KC):
                            nc.tensor.matmul(
                                psq[:],
                                wt[dt][:, hp * 128:(hp + 1) * 128],
                                xin[dt][:, lc * LCH:(lc + 1) * LCH],
                                start=(dt == 0), stop=(dt == KC - 1))
                            yield
                        dst = dest[hp][:, lc * LCH:(lc + 1) * LCH]
                        # copy on ScalarE: keeps the filler chains off the
                        # DVE queue, whose reciprocals would otherwise dam
                        # the chain copy and stall the in-order PE stream
                        if is_q:
                            nc.scalar.activation(dst, psq[:], IDENT,
                                                 bias=bqs[:, hp:hp + 1],
                                                 scale=float(SCALE))
                        else:
                            nc.scalar.activation(dst, psq[:], IDENT,
                                                 bias=bkt[:, hp:hp + 1])
                        yield

                def outproj_gen(dups, hp):
                    for loc in range(2):
                        yield from outproj_one(dups[loc], 2 * hp + loc)

                def run_gen(gen):
                    if gen is not None:
                        for _ in gen:
                            pass

                def epilogue_one(pcp, dup, lc):
                    rc = small.tile([1, LCH], F32, tag="rc",
                                    name="rc", bufs=2)
                    nc.vector.reciprocal(rc[:], pcp[64:65, :])
                    bc = small.tile([64, LCH], F32, tag="bc",
                                    name="bc", bufs=2)
                    nc.gpsimd.partition_broadcast(bc[:], rc[:],
                                                  channels=64)
                    nc.vector.tensor_mul(
                        dup[0:64, lc * LCH:(lc + 1) * LCH],
                        pcp[0:64, :], bc[:])

                def epilogue(pcp_pair, dups, lc):
                    for loc in range(2):
                        epilogue_one(pcp_pair[loc], dups[loc], lc)

                def outproj_one(dup, h):
                    lhs = dup.rearrange("p (r j) -> p j r", j=J)
                    for dc in range(NDC):
                        po = pacc.tile([R, DCH], F32, tag="acc", name="po")
                        for t in range(JE // 128):
                            nc.tensor.matmul(
                                po[:],
                                lhs[:, 2 * t, :],
                                wo_sb[t][:, dc * DCH:(dc + 1) * DCH],
                                start=(t == 0), stop=(t == JE // 128 - 1))
                            yield
                        ob = outp.tile([R, DCH], F32, tag="outp", name="ob")
                        nc.vector.tensor_add(
                            ob[:], po[:],
                            bo_bc[0:R, dc * DCH:(dc + 1) * DCH])
                        nc.sync.dma_start(
                            out_ext[h * R:(h + 1) * R,
                                    dc * DCH:(dc + 1) * DCH],
                            ob[:])
                        yield

                pending = None
                gv = vchain_gen()
                for hp in range(NHP):
                    if hp == 0:
                        # prologue: interleave the two projections through
                        # the (still idle) pacc slots so the accumulation
                        # chains double-buffer instead of serializing on
                        # the PSUM->SBUF copies
                        gk = proj_gen(wkt, kin, kT_sb, 0, False,
                                      pool=pacc, ptag="acc")
                        gq = proj_gen(wqt, qin_t, qT_sb, 0, True,
                                      pool=pacc, ptag="acc")
                        alive = [gk, gq]
                        while alive:
                            for g in list(alive):
                                try:
                                    next(g)
                                except StopIteration:
                                    alive.remove(g)
                        # prime v for the first A@V rounds
                        next(gv)
                        next(gv)
                    fill = deque()
                    if hp + 1 < NHP:
                        fill.append(proj_gen(wkt, kin, kT_sb, hp + 1, False))
                        fill.append(proj_gen(wqt, qin_t, qT_sb, hp + 1,
                                             True))
                    if pending is not None:
                        fill.append(outproj_gen(*pending))
                        pending = None

                    def drain_fill(n, fill=fill):
                        while n > 0 and fill:
                            g = fill[0]
                            try:
                                next(g)
                                n -= 1
                            except StopIteration:
                                fill.popleft()

                    dups = [attnd.tile([128, L], MDT, tag="attnd",
                                       name="dup") for _ in range(2)]
                    ep_prev = None
                    for lc in range(NLC):
                        pavx = [pacc.tile([65, LCH], F32, tag="acc",
                                          name="pavx") for _ in range(2)]
                        for st in range(NST):
                            sc = psc.tile([128, W], F32, tag="psc",
                                          name="sc")
                            for loc in range(2):
                                p0 = loc * 64
                                nc.tensor.matmul(
                                    sc[:, loc * LCH:(loc + 1) * LCH],
                                    kT_sb[hp][p0:p0 + 64,
                                              st * 128:(st + 1) * 128],
                                    qT_sb[hp][p0:p0 + 64,
                                              lc * LCH:(lc + 1) * LCH],
                                    start=True, stop=True)
                            ex = expp.tile([128, W], MDT, tag="exp",
                                           name="ex")
                            nc.scalar.activation(ex[:], sc[:], EXP)
                            if taps and hp == 0 and lc == 0 and st == 0:
                                nc.sync.dma_start(dbg_ex[:], ex[:, 0:LCH])
                            for loc in range(2):
                                h = 2 * hp + loc
                                nc.tensor.matmul(
                                    pavx[loc][:],
                                    v_aug[st][:, h * (E + 1):
                                              (h + 1) * (E + 1)],
                                    ex[:, loc * LCH:(loc + 1) * LCH],
                                    start=(st == 0), stop=(st == NST - 1))
                            if hp == 0 and lc == 0:
                                # drip one v-projection chain per s-tile,
                                # two tiles ahead of its A@V consumer; the
                                # k/q fillers stay out (their input DMAs
                                # may still be in flight)
                                if st + 2 < NST:
                                    try:
                                        next(gv)
                                    except StopIteration:
                                        pass
                            else:
                                drain_fill(fill_per_st)
                        pcp_pair = []
                        for loc in range(2):
                            pcp = small.tile([65, LCH], F32, tag="pcp",
                                             name="pcp", bufs=6)
                            nc.vector.tensor_copy(pcp[:], pavx[loc][:])
                            pcp_pair.append(pcp)
                        # staggered: lc-1's normalize runs while lc+1
                        # accumulates, so reciprocals never gate PSUM reuse
                        if ep_prev is not None:
                            epilogue(ep_prev, dups, lc - 1)
                        ep_prev = pcp_pair
                    if hp + 1 < NHP:
                        epilogue(ep_prev, dups, NLC - 1)
                        for loc in range(2):
                            nc.sync.dma_start(dups[loc][64:128, 0:L - 1],
                                              dups[loc][0:64, 1:L])
                        pending = (dups, hp)
                    else:
                        # last pair: per-head tail so head A's output
                        # projection overlaps head B's epilogue on DVE
                        for loc in range(2):
                            epilogue_one(ep_prev[loc], dups[loc], NLC - 1)
                            nc.sync.dma_start(dups[loc][64:128, 0:L - 1],
                                              dups[loc][0:64, 1:L])
                            run_gen(outproj_one(dups[loc], 2 * hp + loc))
                    if taps and hp == 0:
                        nc.sync.dma_start(dbg_dup[:, 0:L - 1],
                                          dups[0][:, 0:L - 1])
                if pending is not None:
                    run_gen(outproj_gen(*pending))

            if taps:
                for i in range(HEC):
                    nc.sync.dma_start(dbg_qT[i * 128:(i + 1) * 128, :],
                                      qT_sb[i][:])
                    nc.sync.dma_start(dbg_kT[i * 128:(i + 1) * 128, :],
                                      kT_sb[i][:])
                for st in range(NST):
                    nc.sync.dma_start(dbg_v[st * 128:(st + 1) * 128, :],
                                      v_aug[st][:])

    nc.compile()
    return nc


# ---------------------------------------------------------------------------
# host side
# ---------------------------------------------------------------------------

_NC_CACHE = {}

FULL_KEY = (2048, 1024, 8, 1024, "bf16")


def _get_nc(key=FULL_KEY):
    if key not in _NC_CACHE:
        _NC_CACHE[key] = build_core_kernel(*key)
    return _NC_CACHE[key]


def _np_mm_dtype(mm_dt):
    if mm_dt == "bf16":
        import ml_dtypes
        return ml_dtypes.bfloat16
    return np.float32


def make_in_maps(queries, keys, values, Wq, bq, Wk, bk, Wv, bv, Wo, bo,
                 mm_dt="bf16"):
    """Shard: core c handles batch c//2, heads NH*(c%2) .. NH*(c%2)+NH."""
    f = np.float32
    md = _np_mm_dtype(mm_dt)
    half_w = np.asarray(Wq).shape[1] // 2
    in_maps = []
    for c in range(8):
        b, half = c // 2, c % 2
        cs = slice(half * half_w, (half + 1) * half_w)
        in_maps.append({
            "qT": np.ascontiguousarray(np.asarray(queries[b], f).T.astype(md)),
            "kT": np.ascontiguousarray(np.asarray(keys[b], f).T.astype(md)),
            "vT": np.ascontiguousarray(np.asarray(values[b], f).T.astype(md)),
            "wq": np.ascontiguousarray(np.asarray(Wq, f)[:, cs].astype(md)),
            "wk": np.ascontiguousarray(np.asarray(Wk, f)[:, cs].astype(md)),
            "wv": np.ascontiguousarray(np.asarray(Wv, f)[:, cs].astype(md)),
            "bq": np.ascontiguousarray(np.asarray(bq, f)[cs]),
            "bk": np.ascontiguousarray(np.asarray(bk, f)[cs]),
            "bv": np.ascontiguousarray(np.asarray(bv, f)[cs]),
            "wo": np.ascontiguousarray(np.asarray(Wo, f).astype(md)),
            "bo": np.ascontiguousarray(np.asarray(bo, f)),
        })
    return in_maps


def assemble_output(results, B=4, L=2048, OUT_D=1024):
    out = np.empty((B, L, OUT_D), np.float32)
    half_rows = L // 2
    for c in range(8):
        b, half = c // 2, c % 2
        out[b, half * half_rows:(half + 1) * half_rows, :] = results[c]["out"]
    return out


def run_on_hw(inputs, trace=False, key=FULL_KEY, **kw):
    nc = _get_nc(key)
    in_maps = make_in_maps(**inputs, mm_dt=key[4])
    res = run_bass_kernel_spmd(nc, in_maps, core_ids=list(range(8)),
                               trace=trace, **kw)
    return assemble_output(res.results), res


def kernel(**inputs) -> np.ndarray:
    out, _ = run_on_hw(inputs, trace=False)
    return out


# revision 51
# speedup vs baseline: 1.2469x; 1.2469x over previous
"""Trainium2 Bass kernel for the nn_AttentionLayer problem.

Full multi-head attention layer, B=4, L=S=2048, d_model=1024, 16 heads of
dim 64, with the reference's "mix=True" transpose-then-flatten before the
output projection.

Key observation: the mix reshape means output row l' = h*128 + l//16 of each
batch depends ONLY on head h.  So sharding 8 cores as (batch, half-of-heads)
makes every core produce a disjoint, contiguous 1024-row slice of the output
with zero cross-core communication.

Per-core dataflow (all matmul inputs in `mm_dt`, bf16 by default):
  - inputs arrive host-transposed ([D, L]) so the QKV projections contract
    d_model on the partition dim with natural weight layouts
  - v is projected up front into v_aug = [v | ones] s-tiles; the ones
    column makes row 64 of every A@V accumulator the softmax denominator
  - k/q are projected one 128-wide e-chunk (= head pair) at a time; the
    chunk for pair p+1 is drip-fed ("filler" generator) into the PE stream
    while pair p's ACT-bound attention runs, as is pair p-1's output
    projection - the tensor engine fills slack instead of bursting
  - per (pair, l-chunk, s-tile): both heads' scoresT land in one wide
    [128, 1024] PSUM tile via two matmuls on complementary PE row groups
    (true row-packing: K=64 each, concurrent), one wide exp covers both
  - softmax normalize: quick-release copy of each A@V accumulator to SBUF
    (frees the PSUM bank), then reciprocal + gpsimd partition_broadcast +
    multiply, staggered one l-chunk behind the accumulation
  - the normalized attn [64, L] is self-copied (SBUF->SBUF DMA) to
    partitions 64..127 shifted by one position, which makes the output
    projection a clean K=128 matmul against natural Wo row-pair tiles
"""

from collections import deque

import numpy as np

import concourse.bass as bass
import concourse.mybir as mybir
import concourse.tile as tile
from concourse import bacc
from concourse.bass_utils import run_bass_kernel_spmd

F32 = mybir.dt.float32
EXP = mybir.ActivationFunctionType.Exp
IDENT = mybir.ActivationFunctionType.Identity
MULT = mybir.AluOpType.mult
ADD = mybir.AluOpType.add

E = 64          # head dim
J = 16          # mix factor: total heads in the reference model
JE = J * E      # 1024 rows of Wo

MM_DTS = {
    "f32r": mybir.dt.float32r,
    "f32": mybir.dt.float32,
    "bf16": mybir.dt.bfloat16,
}


def build_core_kernel(L=2048, D=1024, NH=8, OUT_D=1024, mm_dt="bf16",
                      taps=False, fill_per_st=3):
    """Builds the per-core Bacc graph (SPMD: all 8 cores run this)."""
    HE = NH * E               # projected width per core
    HEC = HE // 128           # qT/kT tiles (head pairs)
    NHP = NH // 2             # head pairs
    KC = D // 128             # contraction tiles for projections
    R = L // J                # output rows per head
    LCH = min(512, L)         # l-chunk
    NLC = L // LCH
    NST = L // 128            # s-tiles
    DCH = min(512, OUT_D)     # out-proj n-chunk
    NDC = OUT_D // DCH
    SCALE = 1.0 / np.sqrt(E)
    assert L % J == 0 and R <= 128 and HE % 128 == 0

    MDT = MM_DTS[mm_dt]

    nc = bacc.Bacc("TRN2", target_bir_lowering=False, debug=False,
                   enable_asserts=False)

    qT_ext = nc.declare_dram_parameter("qT", [D, L], MDT, isOutput=False)
    kT_ext = nc.declare_dram_parameter("kT", [D, L], MDT, isOutput=False)
    vT_ext = nc.declare_dram_parameter("vT", [D, L], MDT, isOutput=False)
    wq_ext = nc.declare_dram_parameter("wq", [D, HE], MDT, isOutput=False)
    wk_ext = nc.declare_dram_parameter("wk", [D, HE], MDT, isOutput=False)
    wv_ext = nc.declare_dram_parameter("wv", [D, HE], MDT, isOutput=False)
    bq_ext = nc.declare_dram_parameter("bq", [HE], F32, isOutput=False)
    bk_ext = nc.declare_dram_parameter("bk", [HE], F32, isOutput=False)
    bv_ext = nc.declare_dram_parameter("bv", [HE], F32, isOutput=False)
    wo_ext = nc.declare_dram_parameter("wo", [JE, OUT_D], MDT, isOutput=False)
    bo_ext = nc.declare_dram_parameter("bo", [OUT_D], F32, isOutput=False)
    out_ext = nc.declare_dram_parameter("out", [NH * R, OUT_D], F32,
                                        isOutput=True)
    if taps:
        dbg_qT = nc.declare_dram_parameter("dbg_qT", [HEC * 128, L], MDT,
                                           isOutput=True)
        dbg_kT = nc.declare_dram_parameter("dbg_kT", [HEC * 128, L], MDT,
                                           isOutput=True)
        dbg_v = nc.declare_dram_parameter("dbg_v", [NST * 128, NH * (E + 1)],
                                          MDT, isOutput=True)
        dbg_ex = nc.declare_dram_parameter("dbg_ex", [128, LCH], MDT,
                                           isOutput=True)
        dbg_dup = nc.declare_dram_parameter("dbg_dup", [128, L], MDT,
                                            isOutput=True)

    with tile.TileContext(nc) as tc:
        with (
            tc.tile_pool(name="const", bufs=1) as const,
            tc.tile_pool(name="wsl", bufs=2 * KC + 1) as wsl,
            tc.tile_pool(name="qin", bufs=1) as qin,
            tc.tile_pool(name="acts", bufs=1) as acts,
            tc.tile_pool(name="expp", bufs=6) as expp,
            tc.tile_pool(name="attnd", bufs=3) as attnd,
            tc.tile_pool(name="small", bufs=2) as small,
            tc.tile_pool(name="outp", bufs=3) as outp,
        ):
            # ---- constants ----
            bqt = const.tile([128, HEC], F32, tag="bqt")
            nc.sync.dma_start(bqt[:], bq_ext.rearrange("(c p) -> p c", p=128))
            bqs = const.tile([128, HEC], F32, tag="bqs")
            nc.vector.tensor_scalar_mul(bqs[:], bqt[:], float(SCALE))
            bkt = const.tile([128, HEC], F32, tag="bkt")
            nc.sync.dma_start(bkt[:], bk_ext.rearrange("(c p) -> p c", p=128))

            bv_row = const.tile([1, HE], F32, tag="bv_row")
            nc.sync.dma_start(bv_row[:],
                              bv_ext.rearrange("(o he) -> o he", o=1))
            bv_bc = const.tile([128, HE], F32, tag="bv_bc")
            nc.gpsimd.partition_broadcast(bv_bc[:], bv_row[:], channels=128)

            bo_row = const.tile([1, OUT_D], F32, tag="bo_row")
            nc.sync.dma_start(bo_row[:],
                              bo_ext.rearrange("(o d) -> o d", o=1))
            bo_bc = const.tile([128, OUT_D], F32, tag="bo_bc")
            nc.gpsimd.partition_broadcast(bo_bc[:], bo_row[:], channels=128)

            ones_t = const.tile([128, NH], F32, tag="ones_t")
            nc.vector.memset(ones_t[:], 1.0)

            qT_sb = [acts.tile([128, L], MDT, tag=f"qT{i}", name=f"qT_sb{i}")
                     for i in range(HEC)]
            kT_sb = [acts.tile([128, L], MDT, tag=f"kT{i}", name=f"kT_sb{i}")
                     for i in range(HEC)]

            def load_w(w_ext):
                wt = []
                for dt in range(KC):
                    w = wsl.tile([128, HE], MDT, tag="wsl", name="w_t")
                    nc.sync.dma_start(w[:],
                                      w_ext[dt * 128:(dt + 1) * 128, :])
                    wt.append(w)
                return wt

            def load_xin(in_ext, pfx):
                # full-row d-tiles: one big DMA each, live for the phase
                tiles = []
                for dt in range(KC):
                    x = qin.tile([128, L], MDT, tag=f"{pfx}{dt}",
                                 name=f"{pfx}{dt}", bufs=1)
                    nc.sync.dma_start(x[:],
                                      in_ext[dt * 128:(dt + 1) * 128, :])
                    tiles.append(x)
                return tiles

            # ---- v projection up front (A@V consumes every s-tile of v)
            pp_ctx = tc.tile_pool(name="pp", bufs=4, space="PSUM")
            pp = pp_ctx.__enter__()
            wvt = load_w(wv_ext)
            vin = load_xin(vT_ext, "xk")
            v_aug = []
            for st in range(NST):
                v = acts.tile([128, NH * (E + 1)], MDT, tag=f"vaug{st}",
                              name=f"vaug{st}")
                v_aug.append(v)
                nc.vector.tensor_copy(
                    v.rearrange("p (h u) -> p h u", u=E + 1)[:, :, E:E + 1],
                    ones_t.rearrange("p (h o) -> p h o", o=1))
            # two interleaved accumulation chains hide the PSUM RAW latency
            for sp in range(NST // 2):
                ps2 = [pp.tile([128, HE], F32, tag="pp", name=f"psv{i}")
                       for i in range(2)]
                for dt in range(KC):
                    for i in range(2):
                        st = 2 * sp + i
                        nc.tensor.matmul(
                            ps2[i][:],
                            vin[dt][:, st * 128:(st + 1) * 128],
                            wvt[dt][:],
                            start=(dt == 0), stop=(dt == KC - 1))
                for i in range(2):
                    st = 2 * sp + i
                    nc.vector.tensor_add(
                        v_aug[st].rearrange("p (h u) -> p h u",
                                            u=E + 1)[:, :, 0:E],
                        ps2[i].rearrange("p (h e) -> p h e", e=E)[:],
                        bv_bc.rearrange("p (h e) -> p h e", e=E)[:])
            pp_ctx.__exit__(None, None, None)

            wkt = load_w(wk_ext)
            kin = load_xin(kT_ext, "xk")
            wqt = load_w(wq_ext)
            qin_t = load_xin(qT_ext, "xq")

            # Wo preload after the projection-input DMAs in trace order
            wo_sb = []
            for t in range(JE // 128):
                w = const.tile([128, OUT_D], MDT, tag=f"wo{t}",
                               name=f"wo_sb{t}")
                nc.sync.dma_start(w[:], wo_ext[t * 128:(t + 1) * 128, :])
                wo_sb.append(w)

            # ---- merged attention + drip-fed projections/output ----
            W = 2 * LCH
            with (
                tc.tile_pool(name="pq", bufs=1, space="PSUM") as pq,
                tc.tile_pool(name="psc", bufs=2, space="PSUM") as psc,
                tc.tile_pool(name="pacc", bufs=3, space="PSUM") as pacc,
            ):
                def proj_gen(wt, xin, dest, hp, is_q, pool=None,
                             ptag="pq"):
                    """k/q projection of e-chunk hp, one yield per step."""
                    pool = pool or pq
                    for lc in range(NLC):
                        psq = pool.tile([128, LCH], F32, tag=ptag,
                                        name="psq")
                        for dt in range(KC):
                            nc.tensor.matmul(
                                psq[:],
                                wt[dt][:, hp * 128:(hp + 1) * 128],
                                xin[dt][:, lc * LCH:(lc + 1) * LCH],
                                start=(dt == 0), stop=(dt == KC - 1))
                            yield
                        dst = dest[hp][:, lc * LCH:(lc + 1) * LCH]
                        # copy on ScalarE: keeps the filler chains off the
                        # DVE queue, whose reciprocals would otherwise dam
                        # the chain copy and stall the in-order PE stream
                        if is_q:
                            nc.scalar.activation(dst, psq[:], IDENT,
                                                 bias=bqs[:, hp:hp + 1],
                                                 scale=float(SCALE))
                        else:
                            nc.scalar.activation(dst, psq[:], IDENT,
                                                 bias=bkt[:, hp:hp + 1])
                        yield

                def outproj_gen(dups, hp):
                    for loc in range(2):
                        yield from outproj_one(dups[loc], 2 * hp + loc)

                def run_gen(gen):
                    if gen is not None:
                        for _ in gen:
                            pass

                def epilogue_one(pcp, dup, lc):
                    rc = small.tile([1, LCH], F32, tag="rc",
                                    name="rc", bufs=2)
                    nc.vector.reciprocal(rc[:], pcp[64:65, :])
                    bc = small.tile([64, LCH], F32, tag="bc",
                                    name="bc", bufs=2)
                    nc.gpsimd.partition_broadcast(bc[:], rc[:],
                                                  channels=64)
                    nc.vector.tensor_mul(
                        dup[0:64, lc * LCH:(lc + 1) * LCH],
                        pcp[0:64, :], bc[:])

                def epilogue(pcp_pair, dups, lc):
                    for loc in range(2):
                        epilogue_one(pcp_pair[loc], dups[loc], lc)

                def outproj_one(dup, h):
                    lhs = dup.rearrange("p (r j) -> p j r", j=J)
                    for dc in range(NDC):
                        po = pacc.tile([R, DCH], F32, tag="acc", name="po")
                        for t in range(JE // 128):
                            nc.tensor.matmul(
                                po[:],
                                lhs[:, 2 * t, :],
                                wo_sb[t][:, dc * DCH:(dc + 1) * DCH],
                                start=(t == 0), stop=(t == JE // 128 - 1))
                            yield
                        ob = outp.tile([R, DCH], F32, tag="outp", name="ob")
                        nc.vector.tensor_add(
                            ob[:], po[:],
                            bo_bc[0:R, dc * DCH:(dc + 1) * DCH])
                        nc.sync.dma_start(
                            out_ext[h * R:(h + 1) * R,
                                    dc * DCH:(dc + 1) * DCH],
                            ob[:])
                        yield

                pending = None
                for hp in range(NHP):
                    if hp == 0:
                        # prologue: interleave the two projections through
                        # the (still idle) pacc slots so the accumulation
                        # chains double-buffer instead of serializing on
                        # the PSUM->SBUF copies
                        gk = proj_gen(wkt, kin, kT_sb, 0, False,
                                      pool=pacc, ptag="acc")
                        gq = proj_gen(wqt, qin_t, qT_sb, 0, True,
                                      pool=pacc, ptag="acc")
                        alive = [gk, gq]
                        while alive:
                            for g in list(alive):
                                try:
                                    next(g)
                                except StopIteration:
                                    alive.remove(g)
                    fill = deque()
                    if hp + 1 < NHP:
                        fill.append(proj_gen(wkt, kin, kT_sb, hp + 1, False))
                        fill.append(proj_gen(wqt, qin_t, qT_sb, hp + 1,
                                             True))
                    if pending is not None:
                        fill.append(outproj_gen(*pending))
                        pending = None

                    def drain_fill(n, fill=fill):
                        while n > 0 and fill:
                            g = fill[0]
                            try:
                                next(g)
                                n -= 1
                            except StopIteration:
                                fill.popleft()

                    dups = [attnd.tile([128, L], MDT, tag="attnd",
                                       name="dup") for _ in range(2)]
                    ep_prev = None
                    for lc in range(NLC):
                        pavx = [pacc.tile([65, LCH], F32, tag="acc",
                                          name="pavx") for _ in range(2)]
                        for st in range(NST):
                            sc = psc.tile([128, W], F32, tag="psc",
                                          name="sc")
                            for loc in range(2):
                                p0 = loc * 64
                                nc.tensor.matmul(
                                    sc[:, loc * LCH:(loc + 1) * LCH],
                                    kT_sb[hp][p0:p0 + 64,
                                              st * 128:(st + 1) * 128],
                                    qT_sb[hp][p0:p0 + 64,
                                              lc * LCH:(lc + 1) * LCH],
                                    start=True, stop=True)
                            ex = expp.tile([128, W], MDT, tag="exp",
                                           name="ex")
                            nc.scalar.activation(ex[:], sc[:], EXP)
                            if taps and hp == 0 and lc == 0 and st == 0:
                                nc.sync.dma_start(dbg_ex[:], ex[:, 0:LCH])
                            for loc in range(2):
                                h = 2 * hp + loc
                                nc.tensor.matmul(
                                    pavx[loc][:],
                                    v_aug[st][:, h * (E + 1):
                                              (h + 1) * (E + 1)],
                                    ex[:, loc * LCH:(loc + 1) * LCH],
                                    start=(st == 0), stop=(st == NST - 1))
                            # pair0/lc0: input DMAs are still in flight and
                            # the filler matmuls would block the in-order PE
                            # stream on their semaphores
                            if hp > 0 or lc > 0:
                                drain_fill(fill_per_st)
                        pcp_pair = []
                        for loc in range(2):
                            pcp = small.tile([65, LCH], F32, tag="pcp",
                                             name="pcp", bufs=6)
                            nc.vector.tensor_copy(pcp[:], pavx[loc][:])
                            pcp_pair.append(pcp)
                        # staggered: lc-1's normalize runs while lc+1
                        # accumulates, so reciprocals never gate PSUM reuse
                        if ep_prev is not None:
                            epilogue(ep_prev, dups, lc - 1)
                        ep_prev = pcp_pair
                    if hp + 1 < NHP:
                        epilogue(ep_prev, dups, NLC - 1)
                        for loc in range(2):
                            nc.sync.dma_start(dups[loc][64:128, 0:L - 1],
                                              dups[loc][0:64, 1:L])
                        pending = (dups, hp)
                    else:
                        # last pair: per-head tail so head A's output
                        # projection overlaps head B's epilogue on DVE
                        for loc in range(2):
                            epilogue_one(ep_prev[loc], dups[loc], NLC - 1)
                            nc.sync.dma_start(dups[loc][64:128, 0:L - 1],
                                              dups[loc][0:64, 1:L])
                            run_gen(outproj_one(dups[loc], 2 * hp + loc))
                    if taps and hp == 0:
                        nc.sync.dma_start(dbg_dup[:, 0:L - 1],
                                          dups[0][:, 0:L - 1])
                if pending is not None:
                    run_gen(outproj_gen(*pending))

            if taps:
                for i in range(HEC):
                    nc.sync.dma_start(dbg_qT[i * 128:(i + 1) * 128, :],
                                      qT_sb[i][:])
                    nc.sync.dma_start(dbg_kT[i * 128:(i + 1) * 128, :],
                                      kT_sb[i][:])
                for st in range(NST):
                    nc.sync.dma_start(dbg_v[st * 128:(st + 1) * 128, :],
                                      v_aug[st][:])

    nc.compile()
    return nc


# ---------------------------------------------------------------------------
# host side
# ---------------------------------------------------------------------------

_NC_CACHE = {}

FULL_KEY = (2048, 1024, 8, 1024, "bf16")


def _get_nc(key=FULL_KEY):
    if key not in _NC_CACHE:
        _NC_CACHE[key] = build_core_kernel(*key)
    return _NC_CACHE[key]


def _np_mm_dtype(mm_dt):
    if mm_dt == "bf16":
        import ml_dtypes
        return ml_dtypes.bfloat16
    return np.float32


def make_in_maps(queries, keys, values, Wq, bq, Wk, bk, Wv, bv, Wo, bo,
                 mm_dt="bf16"):
    """Shard: core c handles batch c//2, heads NH*(c%2) .. NH*(c%2)+NH."""
    f = np.float32
    md = _np_mm_dtype(mm_dt)
    half_w = np.asarray(Wq).shape[1] // 2
    in_maps = []
    for c in range(8):
        b, half = c // 2, c % 2
        cs = slice(half * half_w, (half + 1) * half_w)
        in_maps.append({
            "qT": np.ascontiguousarray(np.asarray(queries[b], f).T.astype(md)),
            "kT": np.ascontiguousarray(np.asarray(keys[b], f).T.astype(md)),
            "vT": np.ascontiguousarray(np.asarray(values[b], f).T.astype(md)),
            "wq": np.ascontiguousarray(np.asarray(Wq, f)[:, cs].astype(md)),
            "wk": np.ascontiguousarray(np.asarray(Wk, f)[:, cs].astype(md)),
            "wv": np.ascontiguousarray(np.asarray(Wv, f)[:, cs].astype(md)),
            "bq": np.ascontiguousarray(np.asarray(bq, f)[cs]),
            "bk": np.ascontiguousarray(np.asarray(bk, f)[cs]),
            "bv": np.ascontiguousarray(np.asarray(bv, f)[cs]),
            "wo": np.ascontiguousarray(np.asarray(Wo, f).astype(md)),
            "bo": np.ascontiguousarray(np.asarray(bo, f)),
        })
    return in_maps


def assemble_output(results, B=4, L=2048, OUT_D=1024):
    out = np.empty((B, L, OUT_D), np.float32)
    half_rows = L // 2
    for c in range(8):
        b, half = c // 2, c % 2
        out[b, half * half_rows:(half + 1) * half_rows, :] = results[c]["out"]
    return out


def run_on_hw(inputs, trace=False, key=FULL_KEY, **kw):
    nc = _get_nc(key)
    in_maps = make_in_maps(**inputs, mm_dt=key[4])
    res = run_bass_kernel_spmd(nc, in_maps, core_ids=list(range(8)),
                               trace=trace, **kw)
    return assemble_output(res.results), res


def kernel(**inputs) -> np.ndarray:
    out, _ = run_on_hw(inputs, trace=False)
    return out
